# revision 6
# baseline (speedup 1.0000x reference)
"""ACT (Adaptive Computation Time) LSTM cell on 8 TRN2 NeuronCores.

Strategy:
- The reference scans MAX_PONDER=100 steps, but once every row has halted
  the ponder weight p is exactly 0, so the accumulators never change again.
  For the given input distribution all rows halt by step 2 (ponder_count=3),
  so the kernel unrolls S=3 LSTM steps; a host-side numpy fallback continues
  from the kernel's final state in the (never-observed) case rows survive.
- Data-parallel: batch 2048 is sharded 256 rows/core; weights are replicated
  and streamed from HBM in bf16 (halves the bandwidth; fp32 would be
  HBM-bound). All compute-side accumulation is fp32 (PSUM / DVE).
- Transposed layout [feature, row]: matmul contraction dims and per-feature
  biases land on partitions; per-row halting state lives on a single
  partition [1, R] and is broadcast across partitions with a K=1 outer
  product on the TensorEngine.
- aout is NOT accumulated per step: aout = sum_t p_t (hx_t W_o^T + b_o)
  = ahx W_o^T + (sum_t p_t) b_o, with sum_t p_t = final gx_acc.
"""

import numpy as np
import ml_dtypes

B, IN_DIM, HID, OUT = 2048, 2047, 2048, 2048
G4 = 4 * HID            # 8192 gate features
NCORES = 8
R = B // NCORES         # 256 rows per core
S = 3                   # unrolled ponder steps
NK = HID // 128         # 16 contraction chunks
NM = G4 // 128          # 64 gate-feature tiles
NH = NK                 # 16 hidden-feature tiles
NO = OUT // 128         # 16 output-feature tiles
MAX_PONDER = 100
EPS32 = np.float32(0.01)
ONE32 = np.float32(1.0)
THRESH = np.float32(np.float64(1.0) - np.float64(0.01))  # 0.99 as f32, same as ref

BF16 = ml_dtypes.bfloat16

_BUILD_CACHE = {}


def _build():
    """Build the 8-core SPMD Bass graph (same program every core)."""
    import concourse.tile as tile
    from concourse import bacc, mybir

    f32 = mybir.dt.float32
    bf16 = mybir.dt.bfloat16
    AF = mybir.ActivationFunctionType
    OP = mybir.AluOpType

    nc = bacc.Bacc(None)

    # ---- kernel I/O ----
    d_x1t = nc.declare_dram_parameter("x1t", [128, NK, R], bf16, isOutput=False)
    d_hxt = nc.declare_dram_parameter("hxt", [128, NK, R], bf16, isOutput=False)
    d_cxt = nc.declare_dram_parameter("cxt", [128, NK, R], f32, isOutput=False)
    d_wih = nc.declare_dram_parameter("wih", [NM, 128, NK, 128], bf16, isOutput=False)
    d_whh = nc.declare_dram_parameter("whh", [NM, 128, NK, 128], bf16, isOutput=False)
    d_wot = nc.declare_dram_parameter("wot", [NO, 128, NK, 128], bf16, isOutput=False)
    d_wpt = nc.declare_dram_parameter("wpt", [128, NK], bf16, isOutput=False)
    d_bsum = nc.declare_dram_parameter("bsum", [128, NM], f32, isOutput=False)
    d_wcol = nc.declare_dram_parameter("wcol", [128, NM], f32, isOutput=False)
    d_bo = nc.declare_dram_parameter("bo", [1, OUT], bf16, isOutput=False)
    d_bp = nc.declare_dram_parameter("bp", [1, 1], f32, isOutput=False)

    d_ahx = nc.declare_dram_parameter("ahxt", [128, NH, R], f32, isOutput=True)
    d_acx = nc.declare_dram_parameter("acxt", [128, NH, R], f32, isOutput=True)
    d_aout = nc.declare_dram_parameter("aoutt", [128, NO, R], f32, isOutput=True)
    d_hxf = nc.declare_dram_parameter("hxtf", [128, NH, R], bf16, isOutput=True)
    d_cxf = nc.declare_dram_parameter("cxtf", [128, NH, R], f32, isOutput=True)
    d_accs = nc.declare_dram_parameter("accs", [1, S, R], f32, isOutput=True)

    with tile.TileContext(nc) as tc:
        with (
            tc.tile_pool(name="big", bufs=1) as big,           # persistent tensors
            tc.tile_pool(name="wst", bufs=7) as wst,           # weight stripe stream
            tc.tile_pool(name="gat", bufs=2) as gat,           # per-gate activation tiles
            tc.tile_pool(name="hxp", bufs=2) as hxp,           # hx bf16 double buffer
            tc.tile_pool(name="tiny", bufs=1) as tiny,         # [1,R] halting scratch
            tc.tile_pool(name="psg", bufs=5, space="PSUM") as psg,
            tc.tile_pool(name="pss", bufs=1, space="PSUM") as pss,
        ):
            # persistent SBUF tensors
            x1t = big.tile([128, NK, R], bf16, tag="x1t")
            cxt = big.tile([128, NK, R], f32, tag="cxt")
            base = big.tile([128, NM, R], f32, tag="base")
            ahx = big.tile([128, NH, R], f32, tag="ahx")
            acx = big.tile([128, NH, R], f32, tag="acx")
            wpt = big.tile([128, NK], bf16, tag="wpt")
            bsum = big.tile([128, NM], f32, tag="bsum")
            wcol = big.tile([128, NM], f32, tag="wcol")
            bo = big.tile([1, OUT], bf16, tag="bo")
            bp = big.tile([1, 1], f32, tag="bp")
            ones = big.tile([1, 128], f32, tag="ones")
            gacc = big.tile([1, R], f32, tag="gacc")
            accs = big.tile([1, S, R], f32, tag="accs")

            for q in range(4):
                nc.sync.dma_start(x1t[:, 4 * q:4 * (q + 1), :],
                                  d_x1t[:, 4 * q:4 * (q + 1), :])
            nc.sync.dma_start(cxt[:], d_cxt[:])
            nc.sync.dma_start(wpt[:], d_wpt[:])
            nc.sync.dma_start(bsum[:], d_bsum[:])
            nc.sync.dma_start(wcol[:], d_wcol[:])
            nc.sync.dma_start(bo[:], d_bo[:])
            nc.sync.dma_start(bp[:], d_bp[:])
            nc.vector.memset(ones[:], 1.0)
            nc.vector.memset(gacc[:], 0.0)
            nc.gpsimd.memset(ahx[:], 0.0)
            nc.gpsimd.memset(acx[:], 0.0)

            hx_cur = hxp.tile([128, NK, R], bf16, tag="hx")
            nc.sync.dma_start(hx_cur[:], d_hxt[:])
            ahxb = big.tile([128, NH, R], bf16, tag="ahxb")

            # ---- base = W_ih @ [x,1]^T + (b_ih + b_hh)  (fp32, SBUF-resident)
            for m in range(NM):
                w = wst.tile([128, NK, 128], bf16, tag="w")
                if m < 2:
                    nc.sync.dma_start(w[:, :8, :], d_wih[m][:, :8, :])
                    nc.sync.dma_start(w[:, 8:, :], d_wih[m][:, 8:, :])
                else:
                    nc.sync.dma_start(w[:], d_wih[m])
                ps = psg.tile([128, R], f32, tag="ps")
                for k in range(NK):
                    nc.tensor.matmul(ps[:], w[:, k, :], x1t[:, k, :],
                                     start=(k == 0), stop=(k == NK - 1))
                # evacuate + add bias vector (per-partition)
                nc.scalar.activation(base[:, m, :], ps[:], AF.Identity,
                                     bias=bsum[:, m:m + 1])

            # ---- S unrolled ACT/LSTM steps
            for t in range(S):
                hx_next = hxp.tile([128, NK, R], bf16, tag="hx")
                psx = pss.tile([1, R], f32, tag="psx")
                # gates m-tiles ordered (h, h+NH, h+2NH, h+3NH) so the
                # elementwise chain for feature-tile h can start early.
                # The gx partial for hx chunk k is issued one group later so
                # the elementwise chain is off the PE critical path.
                for h in range(NH):
                    gtiles = []
                    for gi in range(4):
                        m = h + gi * NH
                        w = wst.tile([128, NK, 128], bf16, tag="w")
                        nc.sync.dma_start(w[:], d_whh[m])
                        ps = psg.tile([128, R], f32, tag="ps")
                        for k in range(NK):
                            nc.tensor.matmul(ps[:], w[:, k, :], hx_cur[:, k, :],
                                             start=(k == 0), stop=(k == NK - 1))
                        # += base (fp32) in PSUM, then activation into SBUF
                        nc.vector.tensor_tensor(ps[:], ps[:], base[:, m, :], OP.add)
                        g = gat.tile([128, R], f32, tag=f"g{gi}")
                        nc.scalar.activation(
                            g[:], ps[:],
                            AF.Tanh if gi == 2 else AF.Sigmoid)
                        gtiles.append(g)
                    if h > 0:  # gx partial for the previous chunk
                        nc.tensor.matmul(psx[:], wpt[:, h - 1:h], hx_next[:, h - 1, :],
                                         start=(h == 1), stop=False)
                    si, sf, tg, so = gtiles
                    # cx = sf*cx + si*tg ; hx = so*tanh(cx)
                    t1 = gat.tile([128, R], f32, tag="t1")
                    nc.vector.tensor_tensor(t1[:], si[:], tg[:], OP.mult)
                    nc.vector.tensor_tensor(cxt[:, h, :], sf[:], cxt[:, h, :], OP.mult)
                    nc.vector.tensor_tensor(cxt[:, h, :], cxt[:, h, :], t1[:], OP.add)
                    tcx = gat.tile([128, R], f32, tag="tcx")
                    nc.scalar.activation(tcx[:], cxt[:, h, :], AF.Tanh)
                    nc.vector.tensor_tensor(hx_next[:, h, :], so[:], tcx[:], OP.mult)

                # last gx partial closes the accumulation group
                nc.tensor.matmul(psx[:], wpt[:, NK - 1:NK], hx_next[:, NK - 1, :],
                                 start=False, stop=True)
                gx = tiny.tile([1, R], f32, tag="gx")
                nc.scalar.activation(gx[:], psx[:], AF.Sigmoid, bias=bp[0:1, 0:1])

                # halting bookkeeping (exact fp32, matches reference formula)
                hp = tiny.tile([1, R], f32, tag="hp")      # halt_prev
                gm = tiny.tile([1, R], f32, tag="gm")      # gx*(1-halt_prev)
                ap_ = tiny.tile([1, R], f32, tag="ap")     # acc_pre
                ha = tiny.tile([1, R], f32, tag="ha")      # halt_after
                rx = tiny.tile([1, R], f32, tag="rx")
                p = tiny.tile([1, R], f32, tag="p")
                nc.vector.tensor_scalar(hp[:], gacc[:], float(THRESH), None, OP.is_gt)
                nc.vector.tensor_tensor(gm[:], gx[:], hp[:], OP.mult)
                nc.vector.tensor_tensor(gm[:], gx[:], gm[:], OP.subtract)
                nc.vector.tensor_tensor(ap_[:], gacc[:], gm[:], OP.add)
                nc.vector.tensor_scalar(ha[:], ap_[:], float(THRESH), None, OP.is_gt)
                nc.vector.tensor_tensor(ha[:], ha[:], hp[:], OP.subtract)  # halt
                nc.vector.tensor_scalar(rx[:], ap_[:], 1.0, None, OP.subtract)
                nc.vector.tensor_tensor(rx[:], rx[:], ha[:], OP.mult)
                nc.vector.tensor_tensor(p[:], gm[:], rx[:], OP.subtract)
                nc.vector.tensor_tensor(gacc[:], ap_[:], rx[:], OP.subtract)
                nc.vector.tensor_copy(accs[:, t, :], gacc[:])

                # broadcast p across partitions via K=1 outer product
                psb = pss.tile([128, R], f32, tag="psb")
                nc.tensor.matmul(psb[:], ones[0:1, :], p[0:1, :],
                                 start=True, stop=True)
                pb = gat.tile([128, R], f32, tag="pb")
                nc.vector.tensor_copy(pb[:], psb[:])
                pbb = gat.tile([128, R], bf16, tag="pbb")
                nc.scalar.activation(pbb[:], psb[:], AF.Copy)

                # ahx += p*hx ; acx += p*cx.
                # Steps 0..S-2: gpsimd, interleaved per h-group with the
                # t==0 base de-flag subs, both ordered to stay ahead of the
                # next step's per-group consumption (evac-adds read base[m];
                # the cx update WARs against the t3 read of cxt[h]).
                # Step S-1: DVE for the hx part (aout's critical path) with
                # the bf16 cast fused per group.
                last = t == S - 1
                for h in range(NH):
                    if t == 0:
                        for gi in range(4):
                            m = h + gi * NH
                            nc.gpsimd.tensor_scalar_sub(
                                base[:, m, :], base[:, m, :], wcol[:, m:m + 1])
                    t2 = gat.tile([128, R], f32, tag="t2")
                    t3 = gat.tile([128, R], f32, tag="t3")
                    if last:
                        nc.vector.tensor_tensor(t2[:], pbb[:], hx_next[:, h, :], OP.mult)
                        nc.vector.tensor_tensor(ahx[:, h, :], ahx[:, h, :], t2[:], OP.add)
                        nc.vector.tensor_copy(ahxb[:, h, :], ahx[:, h, :])
                    else:
                        nc.gpsimd.tensor_tensor(t2[:], pbb[:], hx_next[:, h, :], OP.mult)
                        nc.gpsimd.tensor_tensor(ahx[:, h, :], ahx[:, h, :], t2[:], OP.add)
                    nc.gpsimd.tensor_tensor(t3[:], pb[:], cxt[:, h, :], OP.mult)
                    nc.gpsimd.tensor_tensor(acx[:, h, :], acx[:, h, :], t3[:], OP.add)

                hx_cur = hx_next

            # ---- aout^T = W_o @ ahx^T + gx_acc (x) b_o
            gaccb = tiny.tile([1, R], bf16, tag="gaccb")
            nc.vector.tensor_copy(gaccb[:], gacc[:])
            for m in range(NO):
                w = wst.tile([128, NK, 128], bf16, tag="w")
                nc.sync.dma_start(w[:], d_wot[m])
                ps = psg.tile([128, R], f32, tag="ps")
                for k in range(NK):
                    nc.tensor.matmul(ps[:], w[:, k, :], ahxb[:, k, :],
                                     start=(k == 0), stop=False)
                nc.tensor.matmul(ps[:], bo[0:1, 128 * m:128 * (m + 1)], gaccb[0:1, :],
                                 start=False, stop=True)
                ao = gat.tile([128, R], f32, tag="g0")
                nc.vector.tensor_copy(ao[:], ps[:])
                nc.sync.dma_start(d_aout[:, m, :], ao[:])

            # ---- outputs
            nc.sync.dma_start(d_ahx[:], ahx[:])
            nc.sync.dma_start(d_acx[:], acx[:])
            nc.sync.dma_start(d_hxf[:], hx_cur[:])
            nc.sync.dma_start(d_cxf[:], cxt[:])
            nc.sync.dma_start(d_accs[:], accs[:])

    nc.finalize()
    return nc


def _prep_inputs(x, hx, cx, W_ih, b_ih, W_hh, b_hh, W_p, b_p, W_o, b_o):
    """Host-side reshape/transpose/cast into per-core in_maps."""
    f32 = np.float32
    x = np.asarray(x, f32)
    x1 = np.empty((B, IN_DIM + 1), f32)
    x1[:, :IN_DIM] = x
    x1[:, IN_DIM] = 1.0
    # [feat, batch] laid out as [p, k, n]
    def t_feat(a):  # [B, HID] -> [128, NK, B]
        return np.ascontiguousarray(
            a.T.reshape(NK, 128, B).transpose(1, 0, 2))
    x1t = t_feat(x1).astype(BF16)
    hxt = t_feat(np.asarray(hx, f32)).astype(BF16)
    cxt = t_feat(np.asarray(cx, f32))

    def t_w(wt, nm):  # W^T [HID, nm*128] -> [nm, 128, NK, 128]
        return np.ascontiguousarray(
            wt.reshape(NK, 128, nm, 128).transpose(2, 1, 0, 3)).astype(BF16)
    wih = t_w(np.asarray(W_ih, f32).T, NM)
    whh = t_w(np.asarray(W_hh, f32).T, NM)
    wot = t_w(np.asarray(W_o, f32).T, NO)
    wpt = np.ascontiguousarray(
        np.asarray(W_p, f32).reshape(NK, 128).T).astype(BF16)
    bsum = np.ascontiguousarray(
        (np.asarray(b_ih, f32) + np.asarray(b_hh, f32)).reshape(NM, 128).T)
    wcol = np.ascontiguousarray(
        np.asarray(W_ih, f32)[:, IN_DIM].reshape(NM, 128).T)
    bo = np.asarray(b_o, f32).reshape(1, OUT).astype(BF16)
    bp = np.asarray(b_p, f32).reshape(1, 1)

    in_maps = []
    for c in range(NCORES):
        sl = slice(c * R, (c + 1) * R)
        in_maps.append({
            "x1t": np.ascontiguousarray(x1t[:, :, sl]),
            "hxt": np.ascontiguousarray(hxt[:, :, sl]),
            "cxt": np.ascontiguousarray(cxt[:, :, sl]),
            "wih": wih, "whh": whh, "wot": wot, "wpt": wpt,
            "bsum": bsum, "wcol": wcol, "bo": bo, "bp": bp,
        })
    return in_maps


def _assemble(results):
    """Gather per-core outputs into full arrays (row-major [B, *])."""
    def undo(key, dt=np.float32):  # [128, nk, R] -> [R, nk*128] rows x feat
        outs = []
        for c in range(NCORES):
            a = np.asarray(results[c][key]).astype(np.float32)
            outs.append(a.transpose(2, 1, 0).reshape(R, -1))
        return np.concatenate(outs, axis=0)
    ahx = undo("ahxt")
    acx = undo("acxt")
    aout = undo("aoutt")
    hxf = undo("hxtf")
    cxf = undo("cxtf")
    accs = np.concatenate(
        [np.asarray(results[c]["accs"]).reshape(S, R) for c in range(NCORES)],
        axis=1)  # [S, B]
    return ahx, acx, aout, hxf, cxf, accs


def _fallback_continue(t0, hx, cx, gx_acc, ahx, acx, aout_hx_terms,
                       x, W_ih, b_ih, W_hh, b_hh, W_p, b_p):
    """Numpy continuation for steps t0..MAX_PONDER-1 (general-input safety).

    aout is handled via its hx-accumulator: caller passes ahx-equivalent
    accumulation and we return extended (ahx, acx, p_sum, hx-weighted sums).
    """
    f32 = np.float32
    x1 = np.concatenate([x, np.zeros((x.shape[0], 1), f32)], axis=1)
    base = x1 @ W_ih.T.astype(f32) + b_ih + b_hh
    halted_step = None
    for t in range(t0, MAX_PONDER):
        gates = base + hx @ W_hh.T
        i, f, g, o = np.split(gates, 4, axis=1)
        def sg(v):
            return (1.0 / (1.0 + np.exp(-v.astype(f32)))).astype(f32)
        cx = sg(f) * cx + sg(i) * np.tanh(g)
        hx = (sg(o) * np.tanh(cx)).astype(f32)
        gx = sg(hx @ W_p.T + b_p)[:, 0]
        hp = (gx_acc > THRESH).astype(f32)
        gm = gx - gx * hp
        ap_ = gx_acc + gm
        ha = (ap_ > THRESH).astype(f32)
        rx = (ap_ - 1.0) * (ha - hp)
        p = gm - rx
        gx_acc = (ap_ - rx).astype(f32)
        ahx = ahx + p[:, None] * hx
        acx = acx + p[:, None] * cx
        if halted_step is None and ha.min() > 0.5:
            halted_step = t
            break
    return ahx, acx, gx_acc, (halted_step if halted_step is not None
                              else MAX_PONDER - 1)


def run(inputs, trace=False):
    """Execute on hardware. Returns (((ahx, acx), aout, ponder_count), exec_ns)."""
    from concourse.bass_utils import run_bass_kernel_spmd

    if "nc" not in _BUILD_CACHE:
        _BUILD_CACHE["nc"] = _build()
    nc = _BUILD_CACHE["nc"]

    in_maps = _prep_inputs(**inputs)
    res = run_bass_kernel_spmd(nc, in_maps, list(range(NCORES)), trace=trace)
    ahx, acx, aout, hxf, cxf, accs = _assemble(res.results)

    # ponder_count: first step (1-indexed) at which every row has halted
    halted = (accs > THRESH).all(axis=1)  # [S]
    if halted.any():
        pc = int(np.argmax(halted)) + 1
    else:
        # inputs outside the expected regime: continue on host in numpy
        f32 = np.float32
        x = np.asarray(inputs["x"], f32)
        gx_acc = accs[S - 1].astype(f32)
        ahx2, acx2, gx_acc2, hstep = _fallback_continue(
            S, hxf.astype(f32), cxf.astype(f32), gx_acc, ahx, acx, None,
            x, np.asarray(inputs["W_ih"], f32), np.asarray(inputs["b_ih"], f32),
            np.asarray(inputs["W_hh"], f32), np.asarray(inputs["b_hh"], f32),
            np.asarray(inputs["W_p"], f32), np.asarray(inputs["b_p"], f32))
        # recompute aout from scratch: aout = ahx @ W_o^T + sum_p * b_o
        W_o = np.asarray(inputs["W_o"], f32)
        b_o = np.asarray(inputs["b_o"], f32)
        aout = ahx2 @ W_o.T + gx_acc2[:, None] * b_o
        ahx, acx = ahx2, acx2
        pc = hstep + 1
    pc = np.int32(pc)
    return ((ahx, acx), aout, pc), res.exec_time_ns


def kernel(**inputs):
    out, _ = run(inputs, trace=False)
    return out


# revision 7
# speedup vs baseline: 1.4428x; 1.4428x over previous
"""ACT (Adaptive Computation Time) LSTM cell on 8 TRN2 NeuronCores.

Strategy:
- The reference scans MAX_PONDER=100 steps, but once every row has halted
  the ponder weight p is exactly 0, so the accumulators never change again.
  For the given input distribution all rows halt by step 2 (ponder_count=3),
  so the kernel unrolls S=3 LSTM steps; a host-side numpy fallback continues
  from the kernel's final state in the (never-observed) case rows survive.
- Data-parallel: batch 2048 is sharded 256 rows/core; weights are replicated
  and streamed from HBM in bf16 (halves the bandwidth; fp32 would be
  HBM-bound). All compute-side accumulation is fp32 (PSUM / DVE).
- Transposed layout [feature, row]: matmul contraction dims and per-feature
  biases land on partitions; per-row halting state lives on a single
  partition [1, R] and is broadcast across partitions with a K=1 outer
  product on the TensorEngine.
- aout is NOT accumulated per step: aout = sum_t p_t (hx_t W_o^T + b_o)
  = ahx W_o^T + (sum_t p_t) b_o, with sum_t p_t = final gx_acc.
"""

import numpy as np
import ml_dtypes

B, IN_DIM, HID, OUT = 2048, 2047, 2048, 2048
G4 = 4 * HID            # 8192 gate features
NCORES = 8
R = B // NCORES         # 256 rows per core
S = 3                   # unrolled ponder steps
NK = HID // 128         # 16 contraction chunks
NM = G4 // 128          # 64 gate-feature tiles
NH = NK                 # 16 hidden-feature tiles
NO = OUT // 128         # 16 output-feature tiles
MAX_PONDER = 100
EPS32 = np.float32(0.01)
ONE32 = np.float32(1.0)
THRESH = np.float32(np.float64(1.0) - np.float64(0.01))  # 0.99 as f32, same as ref

BF16 = ml_dtypes.bfloat16

_BUILD_CACHE = {}


def _build():
    """Build the 8-core SPMD Bass graph (same program every core)."""
    import concourse.tile as tile
    from concourse import bacc, mybir

    f32 = mybir.dt.float32
    bf16 = mybir.dt.bfloat16
    AF = mybir.ActivationFunctionType
    OP = mybir.AluOpType

    nc = bacc.Bacc(None)

    # ---- kernel I/O ----
    d_x1t = nc.declare_dram_parameter("x1t", [128, NK, R], bf16, isOutput=False)
    d_hxt = nc.declare_dram_parameter("hxt", [128, NK, R], bf16, isOutput=False)
    d_cxt = nc.declare_dram_parameter("cxt", [128, NK, R], f32, isOutput=False)
    d_wih = nc.declare_dram_parameter("wih", [NM, 128, NK, 128], bf16, isOutput=False)
    d_whh = nc.declare_dram_parameter("whh", [NM, 128, NK, 128], bf16, isOutput=False)
    d_wot = nc.declare_dram_parameter("wot", [NO, 128, NK, 128], bf16, isOutput=False)
    d_wpt = nc.declare_dram_parameter("wpt", [128, NK], bf16, isOutput=False)
    d_bsum = nc.declare_dram_parameter("bsum", [128, NM], f32, isOutput=False)
    d_wcol = nc.declare_dram_parameter("wcol", [128, NM], f32, isOutput=False)
    d_bo = nc.declare_dram_parameter("bo", [1, OUT], bf16, isOutput=False)
    d_bp = nc.declare_dram_parameter("bp", [1, 1], f32, isOutput=False)

    d_ahx = nc.declare_dram_parameter("ahxt", [128, NH, R], f32, isOutput=True)
    d_acx = nc.declare_dram_parameter("acxt", [128, NH, R], f32, isOutput=True)
    d_aout = nc.declare_dram_parameter("aoutt", [128, NO, R], f32, isOutput=True)
    d_hxf = nc.declare_dram_parameter("hxtf", [128, NH, R], bf16, isOutput=True)
    d_cxf = nc.declare_dram_parameter("cxtf", [128, NH, R], f32, isOutput=True)
    d_accs = nc.declare_dram_parameter("accs", [1, S, R], f32, isOutput=True)

    with tile.TileContext(nc) as tc:
        with (
            tc.tile_pool(name="big", bufs=1) as big,           # persistent tensors
            tc.tile_pool(name="wst", bufs=7) as wst,           # weight stripe stream
            tc.tile_pool(name="gat", bufs=2) as gat,           # per-gate activation tiles
            tc.tile_pool(name="hxp", bufs=2) as hxp,           # hx bf16 double buffer
            tc.tile_pool(name="tiny", bufs=1) as tiny,         # [1,R] halting scratch
            tc.tile_pool(name="psg", bufs=5, space="PSUM") as psg,
            tc.tile_pool(name="pss", bufs=1, space="PSUM") as pss,
        ):
            # persistent SBUF tensors
            x1t = big.tile([128, NK, R], bf16, tag="x1t")
            cxt = big.tile([128, NK, R], f32, tag="cxt")
            base = big.tile([128, NM, R], f32, tag="base")
            ahx = big.tile([128, NH, R], f32, tag="ahx")
            acx = big.tile([128, NH, R], f32, tag="acx")
            wpt = big.tile([128, NK], bf16, tag="wpt")
            bsum = big.tile([128, NM], f32, tag="bsum")
            wcol = big.tile([128, NM], f32, tag="wcol")
            bo = big.tile([1, OUT], bf16, tag="bo")
            bp = big.tile([1, 1], f32, tag="bp")
            ones = big.tile([1, 128], f32, tag="ones")
            gacc = big.tile([1, R], f32, tag="gacc")
            accs = big.tile([1, S, R], f32, tag="accs")

            for q in range(4):
                nc.sync.dma_start(x1t[:, 4 * q:4 * (q + 1), :],
                                  d_x1t[:, 4 * q:4 * (q + 1), :])
            nc.sync.dma_start(cxt[:], d_cxt[:])
            nc.sync.dma_start(wpt[:], d_wpt[:])
            nc.sync.dma_start(bsum[:], d_bsum[:])
            nc.sync.dma_start(wcol[:], d_wcol[:])
            nc.sync.dma_start(bo[:], d_bo[:])
            nc.sync.dma_start(bp[:], d_bp[:])
            nc.vector.memset(ones[:], 1.0)
            nc.vector.memset(gacc[:], 0.0)
            nc.gpsimd.memset(ahx[:], 0.0)
            nc.gpsimd.memset(acx[:], 0.0)

            hx_cur = hxp.tile([128, NK, R], bf16, tag="hx")
            nc.sync.dma_start(hx_cur[:], d_hxt[:])
            ahxb = big.tile([128, NH, R], bf16, tag="ahxb")

            # ---- base = W_ih @ [x,1]^T + (b_ih + b_hh)  (fp32, SBUF-resident)
            for m in range(NM):
                w = wst.tile([128, NK, 128], bf16, tag="w")
                if m < 2:
                    nc.sync.dma_start(w[:, :8, :], d_wih[m][:, :8, :])
                    nc.sync.dma_start(w[:, 8:, :], d_wih[m][:, 8:, :])
                else:
                    nc.sync.dma_start(w[:], d_wih[m])
                ps = psg.tile([128, R], f32, tag="ps")
                for k in range(NK):
                    nc.tensor.matmul(ps[:], w[:, k, :], x1t[:, k, :],
                                     start=(k == 0), stop=(k == NK - 1))
                # evacuate + add bias vector (per-partition)
                nc.scalar.activation(base[:, m, :], ps[:], AF.Identity,
                                     bias=bsum[:, m:m + 1])

            # ---- S unrolled ACT/LSTM steps
            prev = None
            for t in range(S):
                hx_next = hxp.tile([128, NK, R], bf16, tag="hx")
                psx = pss.tile([1, R], f32, tag="psx")
                # gates m-tiles ordered (h, h+NH, h+2NH, h+3NH) so the
                # elementwise chain for feature-tile h can start early.
                # The gx partial for hx chunk k is issued one group later so
                # the elementwise chain is off the PE critical path.
                for h in range(NH):
                    gtiles = []
                    for gi in range(4):
                        m = h + gi * NH
                        if t == 1 and prev is not None:
                            # strip the flag-column term before this group's
                            # evac-add consumes base[m]
                            nc.vector.tensor_scalar_sub(
                                base[:, m, :], base[:, m, :], wcol[:, m:m + 1])
                        w = wst.tile([128, NK, 128], bf16, tag="w")
                        nc.sync.dma_start(w[:], d_whh[m])
                        ps = psg.tile([128, R], f32, tag="ps")
                        for k in range(NK):
                            nc.tensor.matmul(ps[:], w[:, k, :], hx_cur[:, k, :],
                                             start=(k == 0), stop=(k == NK - 1))
                        # += base (fp32) in PSUM, then activation into SBUF
                        nc.vector.tensor_tensor(ps[:], ps[:], base[:, m, :], OP.add)
                        g = gat.tile([128, R], f32, tag=f"g{gi}")
                        nc.scalar.activation(
                            g[:], ps[:],
                            AF.Tanh if gi == 2 else AF.Sigmoid)
                        gtiles.append(g)
                    if h > 0:  # gx partial for the previous chunk
                        nc.tensor.matmul(psx[:], wpt[:, h - 1:h], hx_next[:, h - 1, :],
                                         start=(h == 1), stop=False)
                    if prev is not None:
                        # previous step's accumulation, slotted under this
                        # group's matmul shadow. acx must read cxt[h] BEFORE
                        # this group's cx update overwrites it.
                        p_pb, p_pbb, p_hx = prev
                        t3 = gat.tile([128, R], f32, tag="t3")
                        nc.vector.tensor_tensor(t3[:], p_pb[:], cxt[:, h, :], OP.mult)
                        nc.vector.tensor_tensor(acx[:, h, :], acx[:, h, :], t3[:], OP.add)
                        t2 = gat.tile([128, R], f32, tag="t2")
                        nc.vector.tensor_tensor(t2[:], p_pbb[:], p_hx[:, h, :], OP.mult)
                        nc.vector.tensor_tensor(ahx[:, h, :], ahx[:, h, :], t2[:], OP.add)
                    si, sf, tg, so = gtiles
                    # cx = sf*cx + si*tg ; hx = so*tanh(cx)
                    t1 = gat.tile([128, R], f32, tag="t1")
                    nc.vector.tensor_tensor(t1[:], si[:], tg[:], OP.mult)
                    nc.vector.tensor_tensor(cxt[:, h, :], sf[:], cxt[:, h, :], OP.mult)
                    nc.vector.tensor_tensor(cxt[:, h, :], cxt[:, h, :], t1[:], OP.add)
                    tcx = gat.tile([128, R], f32, tag="tcx")
                    nc.scalar.activation(tcx[:], cxt[:, h, :], AF.Tanh)
                    nc.vector.tensor_tensor(hx_next[:, h, :], so[:], tcx[:], OP.mult)

                # last gx partial closes the accumulation group
                nc.tensor.matmul(psx[:], wpt[:, NK - 1:NK], hx_next[:, NK - 1, :],
                                 start=False, stop=True)
                gx = tiny.tile([1, R], f32, tag="gx")
                nc.scalar.activation(gx[:], psx[:], AF.Sigmoid, bias=bp[0:1, 0:1])

                # halting bookkeeping (exact fp32, matches reference formula)
                hp = tiny.tile([1, R], f32, tag="hp")      # halt_prev
                gm = tiny.tile([1, R], f32, tag="gm")      # gx*(1-halt_prev)
                ap_ = tiny.tile([1, R], f32, tag="ap")     # acc_pre
                ha = tiny.tile([1, R], f32, tag="ha")      # halt_after
                rx = tiny.tile([1, R], f32, tag="rx")
                p = tiny.tile([1, R], f32, tag="p")
                nc.vector.tensor_scalar(hp[:], gacc[:], float(THRESH), None, OP.is_gt)
                nc.vector.tensor_tensor(gm[:], gx[:], hp[:], OP.mult)
                nc.vector.tensor_tensor(gm[:], gx[:], gm[:], OP.subtract)
                nc.vector.tensor_tensor(ap_[:], gacc[:], gm[:], OP.add)
                nc.vector.tensor_scalar(ha[:], ap_[:], float(THRESH), None, OP.is_gt)
                nc.vector.tensor_tensor(ha[:], ha[:], hp[:], OP.subtract)  # halt
                nc.vector.tensor_scalar(rx[:], ap_[:], 1.0, None, OP.subtract)
                nc.vector.tensor_tensor(rx[:], rx[:], ha[:], OP.mult)
                nc.vector.tensor_tensor(p[:], gm[:], rx[:], OP.subtract)
                nc.vector.tensor_tensor(gacc[:], ap_[:], rx[:], OP.subtract)
                nc.vector.tensor_copy(accs[:, t, :], gacc[:])

                # broadcast p across partitions via K=1 outer product
                psb = pss.tile([128, R], f32, tag="psb")
                nc.tensor.matmul(psb[:], ones[0:1, :], p[0:1, :],
                                 start=True, stop=True)
                pb = gat.tile([128, R], f32, tag="pb")
                nc.vector.tensor_copy(pb[:], psb[:])
                pbb = gat.tile([128, R], bf16, tag="pbb")
                nc.scalar.activation(pbb[:], psb[:], AF.Copy)

                if t < S - 1:
                    # defer this step's accumulation into the next step's
                    # h-loop (DVE idle slots under the matmul shadow)
                    prev = (pb, pbb, hx_next)
                else:
                    # last step: hx part + bf16 cast on DVE (aout critical
                    # path); cx part on gpsimd, hidden under the aout phase
                    for h in range(NH):
                        t2 = gat.tile([128, R], f32, tag="t2")
                        nc.vector.tensor_tensor(t2[:], pbb[:], hx_next[:, h, :], OP.mult)
                        nc.vector.tensor_tensor(ahx[:, h, :], ahx[:, h, :], t2[:], OP.add)
                        nc.vector.tensor_copy(ahxb[:, h, :], ahx[:, h, :])
                    for h in range(NH):
                        t3 = gat.tile([128, R], f32, tag="t3")
                        nc.gpsimd.tensor_tensor(t3[:], pb[:], cxt[:, h, :], OP.mult)
                        nc.gpsimd.tensor_tensor(acx[:, h, :], acx[:, h, :], t3[:], OP.add)

                hx_cur = hx_next

            # ---- aout^T = W_o @ ahx^T + gx_acc (x) b_o
            gaccb = tiny.tile([1, R], bf16, tag="gaccb")
            nc.vector.tensor_copy(gaccb[:], gacc[:])
            for m in range(NO):
                w = wst.tile([128, NK, 128], bf16, tag="w")
                nc.sync.dma_start(w[:], d_wot[m])
                ps = psg.tile([128, R], f32, tag="ps")
                for k in range(NK):
                    nc.tensor.matmul(ps[:], w[:, k, :], ahxb[:, k, :],
                                     start=(k == 0), stop=False)
                nc.tensor.matmul(ps[:], bo[0:1, 128 * m:128 * (m + 1)], gaccb[0:1, :],
                                 start=False, stop=True)
                ao = gat.tile([128, R], f32, tag="g0")
                nc.scalar.activation(ao[:], ps[:], AF.Copy)
                nc.sync.dma_start(d_aout[:, m, :], ao[:])

            # ---- outputs
            nc.sync.dma_start(d_ahx[:], ahx[:])
            nc.sync.dma_start(d_acx[:], acx[:])
            nc.sync.dma_start(d_hxf[:], hx_cur[:])
            nc.sync.dma_start(d_cxf[:], cxt[:])
            nc.sync.dma_start(d_accs[:], accs[:])

    nc.finalize()
    return nc


def _prep_inputs(x, hx, cx, W_ih, b_ih, W_hh, b_hh, W_p, b_p, W_o, b_o):
    """Host-side reshape/transpose/cast into per-core in_maps."""
    f32 = np.float32
    x = np.asarray(x, f32)
    x1 = np.empty((B, IN_DIM + 1), f32)
    x1[:, :IN_DIM] = x
    x1[:, IN_DIM] = 1.0
    # [feat, batch] laid out as [p, k, n]
    def t_feat(a):  # [B, HID] -> [128, NK, B]
        return np.ascontiguousarray(
            a.T.reshape(NK, 128, B).transpose(1, 0, 2))
    x1t = t_feat(x1).astype(BF16)
    hxt = t_feat(np.asarray(hx, f32)).astype(BF16)
    cxt = t_feat(np.asarray(cx, f32))

    def t_w(wt, nm):  # W^T [HID, nm*128] -> [nm, 128, NK, 128]
        return np.ascontiguousarray(
            wt.reshape(NK, 128, nm, 128).transpose(2, 1, 0, 3)).astype(BF16)
    wih = t_w(np.asarray(W_ih, f32).T, NM)
    whh = t_w(np.asarray(W_hh, f32).T, NM)
    wot = t_w(np.asarray(W_o, f32).T, NO)
    wpt = np.ascontiguousarray(
        np.asarray(W_p, f32).reshape(NK, 128).T).astype(BF16)
    bsum = np.ascontiguousarray(
        (np.asarray(b_ih, f32) + np.asarray(b_hh, f32)).reshape(NM, 128).T)
    wcol = np.ascontiguousarray(
        np.asarray(W_ih, f32)[:, IN_DIM].reshape(NM, 128).T)
    bo = np.asarray(b_o, f32).reshape(1, OUT).astype(BF16)
    bp = np.asarray(b_p, f32).reshape(1, 1)

    in_maps = []
    for c in range(NCORES):
        sl = slice(c * R, (c + 1) * R)
        in_maps.append({
            "x1t": np.ascontiguousarray(x1t[:, :, sl]),
            "hxt": np.ascontiguousarray(hxt[:, :, sl]),
            "cxt": np.ascontiguousarray(cxt[:, :, sl]),
            "wih": wih, "whh": whh, "wot": wot, "wpt": wpt,
            "bsum": bsum, "wcol": wcol, "bo": bo, "bp": bp,
        })
    return in_maps


def _assemble(results):
    """Gather per-core outputs into full arrays (row-major [B, *])."""
    def undo(key, dt=np.float32):  # [128, nk, R] -> [R, nk*128] rows x feat
        outs = []
        for c in range(NCORES):
            a = np.asarray(results[c][key]).astype(np.float32)
            outs.append(a.transpose(2, 1, 0).reshape(R, -1))
        return np.concatenate(outs, axis=0)
    ahx = undo("ahxt")
    acx = undo("acxt")
    aout = undo("aoutt")
    hxf = undo("hxtf")
    cxf = undo("cxtf")
    accs = np.concatenate(
        [np.asarray(results[c]["accs"]).reshape(S, R) for c in range(NCORES)],
        axis=1)  # [S, B]
    return ahx, acx, aout, hxf, cxf, accs


def _fallback_continue(t0, hx, cx, gx_acc, ahx, acx, aout_hx_terms,
                       x, W_ih, b_ih, W_hh, b_hh, W_p, b_p):
    """Numpy continuation for steps t0..MAX_PONDER-1 (general-input safety).

    aout is handled via its hx-accumulator: caller passes ahx-equivalent
    accumulation and we return extended (ahx, acx, p_sum, hx-weighted sums).
    """
    f32 = np.float32
    x1 = np.concatenate([x, np.zeros((x.shape[0], 1), f32)], axis=1)
    base = x1 @ W_ih.T.astype(f32) + b_ih + b_hh
    halted_step = None
    for t in range(t0, MAX_PONDER):
        gates = base + hx @ W_hh.T
        i, f, g, o = np.split(gates, 4, axis=1)
        def sg(v):
            return (1.0 / (1.0 + np.exp(-v.astype(f32)))).astype(f32)
        cx = sg(f) * cx + sg(i) * np.tanh(g)
        hx = (sg(o) * np.tanh(cx)).astype(f32)
        gx = sg(hx @ W_p.T + b_p)[:, 0]
        hp = (gx_acc > THRESH).astype(f32)
        gm = gx - gx * hp
        ap_ = gx_acc + gm
        ha = (ap_ > THRESH).astype(f32)
        rx = (ap_ - 1.0) * (ha - hp)
        p = gm - rx
        gx_acc = (ap_ - rx).astype(f32)
        ahx = ahx + p[:, None] * hx
        acx = acx + p[:, None] * cx
        if halted_step is None and ha.min() > 0.5:
            halted_step = t
            break
    return ahx, acx, gx_acc, (halted_step if halted_step is not None
                              else MAX_PONDER - 1)


def run(inputs, trace=False):
    """Execute on hardware. Returns (((ahx, acx), aout, ponder_count), exec_ns)."""
    from concourse.bass_utils import run_bass_kernel_spmd

    if "nc" not in _BUILD_CACHE:
        _BUILD_CACHE["nc"] = _build()
    nc = _BUILD_CACHE["nc"]

    in_maps = _prep_inputs(**inputs)
    res = run_bass_kernel_spmd(nc, in_maps, list(range(NCORES)), trace=trace)
    ahx, acx, aout, hxf, cxf, accs = _assemble(res.results)

    # ponder_count: first step (1-indexed) at which every row has halted
    halted = (accs > THRESH).all(axis=1)  # [S]
    if halted.any():
        pc = int(np.argmax(halted)) + 1
    else:
        # inputs outside the expected regime: continue on host in numpy
        f32 = np.float32
        x = np.asarray(inputs["x"], f32)
        gx_acc = accs[S - 1].astype(f32)
        ahx2, acx2, gx_acc2, hstep = _fallback_continue(
            S, hxf.astype(f32), cxf.astype(f32), gx_acc, ahx, acx, None,
            x, np.asarray(inputs["W_ih"], f32), np.asarray(inputs["b_ih"], f32),
            np.asarray(inputs["W_hh"], f32), np.asarray(inputs["b_hh"], f32),
            np.asarray(inputs["W_p"], f32), np.asarray(inputs["b_p"], f32))
        # recompute aout from scratch: aout = ahx @ W_o^T + sum_p * b_o
        W_o = np.asarray(inputs["W_o"], f32)
        b_o = np.asarray(inputs["b_o"], f32)
        aout = ahx2 @ W_o.T + gx_acc2[:, None] * b_o
        ahx, acx = ahx2, acx2
        pc = hstep + 1
    pc = np.int32(pc)
    return ((ahx, acx), aout, pc), res.exec_time_ns


def kernel(**inputs):
    out, _ = run(inputs, trace=False)
    return out
